# revision 23
# baseline (speedup 1.0000x reference)
"""Trainium2 Bass kernel for nn_BC_5274219839877.

Computes, for b=64, n_v=128, n_q=32, d_v=2048, d_q=1024, K=3072, H=8:
    v_ = relu((v_g/||v_w||) * v @ v_w^T + v_b)        [b, n_v, K]
    q_ = relu((q_g/||q_w||) * q @ q_w^T + q_b)        [b, n_q, K]
    out[b,h,i,j] = sum_k hm[h,k] v_[b,i,k] q_[b,j,k] + h_bias[h]

Sharding: data-parallel over batch across 8 NeuronCores (8 batches/core),
weights replicated.  All matmul operands are bf16 (norm-scales folded into
the weights host-side); the PE streams 1 column/cycle regardless of dtype,
so bf16 buys DMA bandwidth and removes the f32->f32r conversion copies.
The per-core PE floor is 491,520 column-cycles (~205us at 2.4GHz); the
steady-state stream here runs gapless at 216ns per 512-col matmul (silicon
peak), so the remaining wall time is the DMA-bound prologue (covered by
HAM-warm dummy matmuls), boot, and the eviction tail.

Key structure:
 - Stage-3 partial products accumulate directly in PSUM across all 24
   k-blocks (2 batch accumulators per 2KB bank; start=True only on each
   bank's first write - the whole-bank has_written clear happens once and
   the second accumulator's first matmul overwrites-where-clear).
 - The q side runs 2 k-blocks ahead; stage 3 lags 1 k-block so its
   matmuls never head-of-line-block the PE queue on scalar/vector
   producers.
 - Weight prefetch DMAs are emitted at the BOTTOM of the iteration with
   minimal ring depths, so Tile's WAR gate (against already-emitted
   readers) paces them and they never flood the prologue wire.
"""

import numpy as np

import concourse.bass as bass
import concourse.tile as tile
from concourse import bacc, mybir
from concourse.bass_utils import run_bass_kernel_spmd

F32 = mybir.dt.float32
BF16 = mybir.dt.bfloat16

N_CORES = 8
B = 64
B_LOC = B // N_CORES       # 8 batches per core
NV = 128
NQ = 32
DV = 2048
DQ = 1024
K = 3072
H = 8

KB = 128                   # k-block size (PSUM partition dim)
NKB = K // KB              # 24 k-blocks
TV = DV // 128             # 16 d-tiles (v side)
TQ = DQ // 128             # 8 d-tiles (q side)
MV = B_LOC * NV            # 1024
MQ = B_LOC * NQ            # 256

WARM = 118                 # HAM-warm dummy matmuls covering the DMA prologue
QA = 2                     # q-side runs QA k-blocks ahead of v-side
QXD = 4                    # qx ring depth
WVD = 2                    # wv ring depth (bottom-emitted => WAR-gated)
WQD = 3                    # wq ring depth (same)

_CACHE = {}


def _build_program():
    nc = bacc.Bacc("TRN2", target_bir_lowering=False, debug=False,
                   num_devices=N_CORES)

    vt_d = nc.dram_tensor("vt", [TV // 4, 128, 4 * MV], BF16,
                          kind="ExternalInput")
    qt_d = nc.dram_tensor("qt", [TQ // 4, 128, 4 * MQ], BF16,
                          kind="ExternalInput")
    wv_d = nc.dram_tensor("wv", [NKB, 128, TV * KB], BF16,
                          kind="ExternalInput")
    wq_d = nc.dram_tensor("wq", [NKB, 128, TQ * KB], BF16,
                          kind="ExternalInput")
    msb_d = nc.dram_tensor("msb", [128, NKB * H], F32, kind="ExternalInput")
    vb_d = nc.dram_tensor("vb", [128, NKB], F32, kind="ExternalInput")
    qb_d = nc.dram_tensor("qb", [128, NKB], F32, kind="ExternalInput")
    bias_d = nc.dram_tensor("bias", [128, 2 * H * NQ], F32,
                            kind="ExternalInput")
    out_d = nc.dram_tensor("out", [B_LOC // 2, 128, 2 * H * NQ], BF16,
                           kind="ExternalOutput")

    relu = mybir.ActivationFunctionType.Relu

    msb = nc.alloc_sbuf_tensor("msb_s", [128, NKB * H], F32).ap()
    vb = nc.alloc_sbuf_tensor("vb_s", [128, NKB], F32).ap()
    qb = nc.alloc_sbuf_tensor("qb_s", [128, NKB], F32).ap()
    bias = nc.alloc_sbuf_tensor("bias_s", [128, 2 * H * NQ], F32).ap()
    vt_big = nc.alloc_sbuf_tensor("vts", [128, TV * MV], BF16).ap()
    vt = [vt_big[:, t * MV:(t + 1) * MV] for t in range(TV)]
    qt_big = nc.alloc_sbuf_tensor("qts", [128, TQ * MQ], BF16).ap()
    qt = [qt_big[:, t * MQ:(t + 1) * MQ] for t in range(TQ)]
    wv_s = [nc.alloc_sbuf_tensor(f"wvs{i}", [128, TV * KB], BF16).ap()
            for i in range(WVD)]
    wq_s = [nc.alloc_sbuf_tensor(f"wqs{i}", [128, TQ * KB], BF16).ap()
            for i in range(WQD)]
    vk = [nc.alloc_sbuf_tensor(f"vk{i}", [128, MV], BF16).ap()
          for i in range(2)]
    qk = [nc.alloc_sbuf_tensor(f"qk{i}", [128, MQ], BF16).ap()
          for i in range(2)]
    qx = [nc.alloc_sbuf_tensor(f"qx{i}", [128, H * MQ], BF16).ap()
          for i in range(QXD)]
    outs = [nc.alloc_sbuf_tensor(f"outs{i}", [128, 2 * H * NQ], BF16).ap()
            for i in range(B_LOC // 2)]
    warm = nc.alloc_sbuf_tensor("warm", [128, 256], BF16).ap()

    psv = [nc.alloc_psum_tensor(f"psv{i}", [128, 512], F32).ap()
           for i in range(3)]
    psq = nc.alloc_psum_tensor("psq", [128, MQ], F32).ap()
    ps3 = [nc.alloc_psum_tensor(f"ps3{i}", [128, 2 * H * NQ], F32).ap()
           for i in range(B_LOC // 2)]

    with tile.TileContext(nc) as tc:
        def vt_dma(c):
            return (vt_big[:, c * 4 * MV:(c + 1) * 4 * MV], vt_d[c])

        # Max ring parallelism: concurrent DMA rings raise aggregate
        # throughput (~500 GB/s); everything lands together ~20us in and
        # the warm matmuls cover the window.
        # Max ring parallelism: concurrent DMA rings raise aggregate
        # throughput (~500 GB/s); everything lands together ~20us in and
        # the warm matmuls cover the window.
        nc.vector.memset(warm, 0.0)
        nc.sync.dma_start(wv_s[0], wv_d[0])
        nc.scalar.dma_start(*vt_dma(1))
        nc.gpsimd.dma_start(*vt_dma(3))
        nc.sync.dma_start(*vt_dma(0))
        nc.scalar.dma_start(*vt_dma(2))
        nc.gpsimd.dma_start(vb, vb_d.ap())
        nc.gpsimd.dma_start(qb, qb_d.ap())
        for c in range(TQ // 4):
            nc.sync.dma_start(qt_big[:, c * 4 * MQ:(c + 1) * 4 * MQ],
                              qt_d[c])
        for i in range(WQD):
            nc.gpsimd.dma_start(wq_s[i], wq_d[i])
        nc.sync.dma_start(wv_s[1], wv_d[1])
        nc.gpsimd.dma_start(msb, msb_d.ap())
        nc.gpsimd.dma_start(bias, bias_d.ap())

        # HAM pre-warm: dummy matmuls on zeros while the prologue DMAs
        # stream, so the real matmul stream starts at the warm PE clock
        for i in range(WARM):
            nc.tensor.matmul(psv[0][:, :256], warm[:, :128], warm,
                             start=True, stop=True)

        def q_side(kb):
            # q_: qk[k, (b,j)] = relu(qt @ wq + qb); qx[k,(b,h,j)] = hm*qk
            # (one PSUM bank: the chain and its activation never overlap
            # the next k-block's chain, 8.7us apart)
            qps = psq
            wqb = wq_s[kb % WQD]
            for t in range(TQ):
                nc.tensor.matmul(
                    qps[:],
                    wqb[:, t * KB:(t + 1) * KB],
                    qt[t],
                    start=(t == 0), stop=(t == TQ - 1))
            qkb = qk[kb % 2]
            nc.scalar.activation(qkb[:], qps[:], relu,
                                 bias=qb[:, kb:kb + 1])
            qxb = qx[kb % QXD]
            qx4 = qxb.rearrange("p (b h j) -> p b h j", b=B_LOC, h=H)
            qk3 = qkb.rearrange("p (b j) -> p b j", b=B_LOC)
            for h in range(H):
                nc.vector.tensor_scalar_mul(
                    qx4[:, :, h, :], qk3[:, :, :],
                    msb[:, kb * H + h:kb * H + h + 1])

        def stage3(kb, evict=False):
            # ps3[b//2][:, (b%2)*256:...] += vk[:, b].T @ qx[:, b, :, :]
            vkb = vk[kb % 2]
            qxb = qx[kb % QXD]
            for b_ in range(B_LOC):
                nc.tensor.matmul(
                    ps3[b_ // 2][:, (b_ % 2) * H * NQ:(b_ % 2 + 1) * H * NQ],
                    vkb[:, b_ * NV:(b_ + 1) * NV],
                    qxb[:, b_ * H * NQ:(b_ + 1) * H * NQ],
                    start=(kb == 0 and b_ % 2 == 0),
                    stop=(kb == NKB - 1),
                    skip_group_check=True)
                if evict and b_ % 2 == 1:
                    p = b_ // 2
                    nc.vector.tensor_add(outs[p][:], ps3[p][:], bias[:])
                    nc.sync.dma_start(out_d[p], outs[p][:])

        # --- k-blocked fused pipeline ---
        for kb in range(NKB):
            wvb = wv_s[kb % WVD]

            # stage 3 first: its inputs (vk/qx of kb-1) completed early in
            # the previous iteration, and leading with it means the final
            # stage-3 is the only work after the last v-chain drains
            if kb >= 1:
                stage3(kb - 1)

            # stage 1 (v): vk[k, m] = relu((v @ v_w^T)^T + v_b)
            vkb = vk[kb % 2]
            for mc in range(MV // 512):
                ps = psv[(2 * kb + mc) % 3]
                for t in range(TV):
                    nc.tensor.matmul(
                        ps[:],
                        wvb[:, t * KB:(t + 1) * KB],
                        vt[t][:, mc * 512:(mc + 1) * 512],
                        start=(t == 0), stop=(t == TV - 1))
                nc.scalar.activation(
                    vkb[:, mc * 512:(mc + 1) * 512], ps[:], relu,
                    bias=vb[:, kb:kb + 1])

            if kb == 0:
                q_side(0)
                q_side(1)
            if kb + QA < NKB:
                q_side(kb + QA)

            # weight prefetches at the bottom of the iteration: the slot's
            # same-ring readers are already emitted, so the Tile WAR gate
            # holds each DMA until they complete (no prologue flood, no
            # clobbering of the weights this iteration is reading)
            if kb + 2 < NKB:
                nc.sync.dma_start(wv_s[(kb + 2) % WVD], wv_d[kb + 2])
            if kb + 3 < NKB:
                nc.gpsimd.dma_start(wq_s[(kb + 3) % WQD], wq_d[kb + 3])

        # final k-block: accumulators evict (+h_bias) as their bank closes
        stage3(NKB - 1, evict=True)

    nc.compile()
    return nc


def _prep_host(inputs):
    bf16 = mybir.dt.np(BF16)
    v = np.asarray(inputs["v"], dtype=np.float32)
    q = np.asarray(inputs["q"], dtype=np.float32)
    v_w = np.asarray(inputs["v_w"], dtype=np.float32)
    q_w = np.asarray(inputs["q_w"], dtype=np.float32)
    v_g = float(np.asarray(inputs["v_g"], dtype=np.float32))
    q_g = float(np.asarray(inputs["q_g"], dtype=np.float32))
    v_b = np.asarray(inputs["v_b"], dtype=np.float32)
    q_b = np.asarray(inputs["q_b"], dtype=np.float32)
    h_mat = np.asarray(inputs["h_mat"], dtype=np.float32)
    h_bias = np.asarray(inputs["h_bias"], dtype=np.float32)

    s_v = v_g / float(np.linalg.norm(v_w))
    s_q = q_g / float(np.linalg.norm(q_w))

    wv_r = np.ascontiguousarray(
        (s_v * v_w).astype(bf16)
        .reshape(NKB, KB, TV, 128).transpose(0, 3, 2, 1)
        .reshape(NKB, 128, TV * KB))
    wq_r = np.ascontiguousarray(
        (s_q * q_w).astype(bf16)
        .reshape(NKB, KB, TQ, 128).transpose(0, 3, 2, 1)
        .reshape(NKB, 128, TQ * KB))
    hm = h_mat[0, :, 0, :]                       # [H, K]
    msb = np.ascontiguousarray(
        hm.T.reshape(NKB, 128, H).transpose(1, 0, 2)
        .reshape(128, NKB * H))
    vb_r = np.ascontiguousarray(v_b.reshape(NKB, 128).T)
    qb_r = np.ascontiguousarray(q_b.reshape(NKB, 128).T)
    hb = h_bias[0, :, 0, 0]                      # [H]
    bias = np.ascontiguousarray(np.broadcast_to(
        np.tile(np.repeat(hb, NQ), 2)[None, :], (128, 2 * H * NQ)))

    in_maps = []
    for c in range(N_CORES):
        vc = v[c * B_LOC:(c + 1) * B_LOC].astype(bf16)
        qc = q[c * B_LOC:(c + 1) * B_LOC].astype(bf16)
        vt_c = np.ascontiguousarray(
            vc.reshape(B_LOC, NV, TV, 128).transpose(2, 3, 0, 1)
            .reshape(TV // 4, 4, 128, MV).transpose(0, 2, 1, 3)
            .reshape(TV // 4, 128, 4 * MV))
        qt_c = np.ascontiguousarray(
            qc.reshape(B_LOC, NQ, TQ, 128).transpose(2, 3, 0, 1)
            .reshape(TQ // 4, 4, 128, MQ).transpose(0, 2, 1, 3)
            .reshape(TQ // 4, 128, 4 * MQ))
        in_maps.append({
            "vt": vt_c, "qt": qt_c, "wv": wv_r, "wq": wq_r,
            "msb": msb, "vb": vb_r, "qb": qb_r, "bias": bias,
        })
    return in_maps


def _run(inputs, trace=False):
    if "nc" not in _CACHE:
        _CACHE["nc"] = _build_program()
    nc = _CACHE["nc"]
    in_maps = _prep_host(inputs)
    res = run_bass_kernel_spmd(nc, in_maps, list(range(N_CORES)), trace=trace)
    out = np.empty((B, H, NV, NQ), dtype=np.float32)
    for c in range(N_CORES):
        oc = np.asarray(res.results[c]["out"], dtype=np.float32)
        out[c * B_LOC:(c + 1) * B_LOC] = (
            oc.reshape(4, NV, 2, H, NQ).transpose(0, 2, 3, 1, 4)
            .reshape(B_LOC, H, NV, NQ))
    return out, res


def kernel(**inputs):
    return _run(inputs)[0]


# revision 24
# speedup vs baseline: 1.0017x; 1.0017x over previous
"""Trainium2 Bass kernel for nn_BC_5274219839877.

Computes, for b=64, n_v=128, n_q=32, d_v=2048, d_q=1024, K=3072, H=8:
    v_ = relu((v_g/||v_w||) * v @ v_w^T + v_b)        [b, n_v, K]
    q_ = relu((q_g/||q_w||) * q @ q_w^T + q_b)        [b, n_q, K]
    out[b,h,i,j] = sum_k hm[h,k] v_[b,i,k] q_[b,j,k] + h_bias[h]

Sharding: data-parallel over batch across 8 NeuronCores (8 batches/core),
weights replicated.  All matmul operands are bf16 (norm-scales folded into
the weights host-side); the PE streams 1 column/cycle regardless of dtype,
so bf16 buys DMA bandwidth and removes the f32->f32r conversion copies.
The per-core PE floor is 491,520 column-cycles (~205us at 2.4GHz); the
steady-state stream here runs gapless at 216ns per 512-col matmul (silicon
peak), so the remaining wall time is the DMA-bound prologue (covered by
HAM-warm dummy matmuls), boot, and the eviction tail.

Key structure:
 - Stage-3 partial products accumulate directly in PSUM across all 24
   k-blocks (2 batch accumulators per 2KB bank; start=True only on each
   bank's first write - the whole-bank has_written clear happens once and
   the second accumulator's first matmul overwrites-where-clear).
 - The q side runs 2 k-blocks ahead; stage 3 lags 1 k-block so its
   matmuls never head-of-line-block the PE queue on scalar/vector
   producers.
 - Weight prefetch DMAs are emitted at the BOTTOM of the iteration with
   minimal ring depths, so Tile's WAR gate (against already-emitted
   readers) paces them and they never flood the prologue wire.
"""

import numpy as np

import concourse.bass as bass
import concourse.tile as tile
from concourse import bacc, mybir
from concourse.bass_utils import run_bass_kernel_spmd

F32 = mybir.dt.float32
BF16 = mybir.dt.bfloat16

N_CORES = 8
B = 64
B_LOC = B // N_CORES       # 8 batches per core
NV = 128
NQ = 32
DV = 2048
DQ = 1024
K = 3072
H = 8

KB = 128                   # k-block size (PSUM partition dim)
NKB = K // KB              # 24 k-blocks
TV = DV // 128             # 16 d-tiles (v side)
TQ = DQ // 128             # 8 d-tiles (q side)
MV = B_LOC * NV            # 1024
MQ = B_LOC * NQ            # 256

WARM = 118                 # HAM-warm dummy matmuls covering the DMA prologue
QA = 2                     # q-side runs QA k-blocks ahead of v-side
QXD = 4                    # qx ring depth
WVD = 2                    # wv ring depth (bottom-emitted => WAR-gated)
WQD = 3                    # wq ring depth (same)

_CACHE = {}


def _build_program():
    nc = bacc.Bacc("TRN2", target_bir_lowering=False, debug=False,
                   num_devices=N_CORES)

    vt_d = nc.dram_tensor("vt", [TV // 4, 128, 4 * MV], BF16,
                          kind="ExternalInput")
    qt_d = nc.dram_tensor("qt", [TQ // 4, 128, 4 * MQ], BF16,
                          kind="ExternalInput")
    wv_d = nc.dram_tensor("wv", [NKB, 128, TV * KB], BF16,
                          kind="ExternalInput")
    wq_d = nc.dram_tensor("wq", [NKB, 128, TQ * KB], BF16,
                          kind="ExternalInput")
    msb_d = nc.dram_tensor("msb", [128, NKB * H], F32, kind="ExternalInput")
    vb_d = nc.dram_tensor("vb", [128, NKB], F32, kind="ExternalInput")
    qb_d = nc.dram_tensor("qb", [128, NKB], F32, kind="ExternalInput")
    bias_d = nc.dram_tensor("bias", [128, 2 * H * NQ], F32,
                            kind="ExternalInput")
    out_d = nc.dram_tensor("out", [B_LOC // 2, 128, 2 * H * NQ], BF16,
                           kind="ExternalOutput")

    relu = mybir.ActivationFunctionType.Relu

    msb = nc.alloc_sbuf_tensor("msb_s", [128, NKB * H], F32).ap()
    vb = nc.alloc_sbuf_tensor("vb_s", [128, NKB], F32).ap()
    qb = nc.alloc_sbuf_tensor("qb_s", [128, NKB], F32).ap()
    bias = nc.alloc_sbuf_tensor("bias_s", [128, 2 * H * NQ], F32).ap()
    vt_big = nc.alloc_sbuf_tensor("vts", [128, TV * MV], BF16).ap()
    vt = [vt_big[:, t * MV:(t + 1) * MV] for t in range(TV)]
    qt_big = nc.alloc_sbuf_tensor("qts", [128, TQ * MQ], BF16).ap()
    qt = [qt_big[:, t * MQ:(t + 1) * MQ] for t in range(TQ)]
    wv_s = [nc.alloc_sbuf_tensor(f"wvs{i}", [128, TV * KB], BF16).ap()
            for i in range(WVD)]
    wq_s = [nc.alloc_sbuf_tensor(f"wqs{i}", [128, TQ * KB], BF16).ap()
            for i in range(WQD)]
    vk = [nc.alloc_sbuf_tensor(f"vk{i}", [128, MV], BF16).ap()
          for i in range(2)]
    qk = [nc.alloc_sbuf_tensor(f"qk{i}", [128, MQ], BF16).ap()
          for i in range(2)]
    qx = [nc.alloc_sbuf_tensor(f"qx{i}", [128, H * MQ], BF16).ap()
          for i in range(QXD)]
    outs = [nc.alloc_sbuf_tensor(f"outs{i}", [128, 2 * H * NQ], BF16).ap()
            for i in range(B_LOC // 2)]
    warm = nc.alloc_sbuf_tensor("warm", [128, 256], BF16).ap()

    psv = [nc.alloc_psum_tensor(f"psv{i}", [128, 512], F32).ap()
           for i in range(3)]
    psq = nc.alloc_psum_tensor("psq", [128, MQ], F32).ap()
    ps3 = [nc.alloc_psum_tensor(f"ps3{i}", [128, 2 * H * NQ], F32).ap()
           for i in range(B_LOC // 2)]

    with tile.TileContext(nc) as tc:
        def vt_dma(c):
            return (vt_big[:, c * 4 * MV:(c + 1) * 4 * MV], vt_d[c])

        # Max ring parallelism: concurrent DMA rings raise aggregate
        # throughput (~500 GB/s); everything lands together ~20us in and
        # the warm matmuls cover the window.
        # Max ring parallelism: concurrent DMA rings raise aggregate
        # throughput (~500 GB/s); everything lands together ~20us in and
        # the warm matmuls cover the window.
        nc.vector.memset(warm, 0.0)
        nc.sync.dma_start(wv_s[0], wv_d[0])
        nc.scalar.dma_start(*vt_dma(1))
        nc.gpsimd.dma_start(*vt_dma(3))
        nc.sync.dma_start(*vt_dma(0))
        nc.scalar.dma_start(*vt_dma(2))
        nc.gpsimd.dma_start(vb, vb_d.ap())
        nc.gpsimd.dma_start(qb, qb_d.ap())
        for c in range(TQ // 4):
            nc.sync.dma_start(qt_big[:, c * 4 * MQ:(c + 1) * 4 * MQ],
                              qt_d[c])
        for i in range(WQD):
            nc.gpsimd.dma_start(wq_s[i], wq_d[i])
        nc.sync.dma_start(wv_s[1], wv_d[1])
        nc.gpsimd.dma_start(msb, msb_d.ap())
        nc.gpsimd.dma_start(bias, bias_d.ap())

        # HAM pre-warm: dummy matmuls on zeros while the prologue DMAs
        # stream, so the real matmul stream starts at the warm PE clock
        for i in range(WARM):
            nc.tensor.matmul(psv[0][:, :256], warm[:, :128], warm,
                             start=True, stop=True)

        def q_side(kb):
            # q_: qk[k, (b,j)] = relu(qt @ wq + qb); qx[k,(b,h,j)] = hm*qk
            # (one PSUM bank: the chain and its activation never overlap
            # the next k-block's chain, 8.7us apart)
            qps = psq
            wqb = wq_s[kb % WQD]
            for t in range(TQ):
                nc.tensor.matmul(
                    qps[:],
                    wqb[:, t * KB:(t + 1) * KB],
                    qt[t],
                    start=(t == 0), stop=(t == TQ - 1))
            qkb = qk[kb % 2]
            nc.scalar.activation(qkb[:], qps[:], relu,
                                 bias=qb[:, kb:kb + 1])
            qxb = qx[kb % QXD]
            qx4 = qxb.rearrange("p (b h j) -> p b h j", b=B_LOC, h=H)
            qk3 = qkb.rearrange("p (b j) -> p b j", b=B_LOC)
            for h in range(H):
                nc.vector.tensor_scalar_mul(
                    qx4[:, :, h, :], qk3[:, :, :],
                    msb[:, kb * H + h:kb * H + h + 1])

        def stage3(kb, evict=False):
            # ps3[b//2][:, (b%2)*256:...] += vk[:, b].T @ qx[:, b, :, :]
            vkb = vk[kb % 2]
            qxb = qx[kb % QXD]
            for b_ in range(B_LOC):
                nc.tensor.matmul(
                    ps3[b_ // 2][:, (b_ % 2) * H * NQ:(b_ % 2 + 1) * H * NQ],
                    vkb[:, b_ * NV:(b_ + 1) * NV],
                    qxb[:, b_ * H * NQ:(b_ + 1) * H * NQ],
                    start=(kb == 0 and b_ % 2 == 0),
                    stop=(kb == NKB - 1),
                    skip_group_check=True)
                if evict and b_ % 2 == 1:
                    p = b_ // 2
                    nc.vector.tensor_add(outs[p][:], ps3[p][:], bias[:])
                    nc.sync.dma_start(out_d[p], outs[p][:])

        # --- k-blocked fused pipeline ---
        for kb in range(NKB):
            wvb = wv_s[kb % WVD]

            # stage 1 (v): vk[k, m] = relu((v @ v_w^T)^T + v_b)
            vkb = vk[kb % 2]
            for mc in range(MV // 512):
                ps = psv[(2 * kb + mc) % 3]
                for t in range(TV):
                    nc.tensor.matmul(
                        ps[:],
                        wvb[:, t * KB:(t + 1) * KB],
                        vt[t][:, mc * 512:(mc + 1) * 512],
                        start=(t == 0), stop=(t == TV - 1))
                nc.scalar.activation(
                    vkb[:, mc * 512:(mc + 1) * 512], ps[:], relu,
                    bias=vb[:, kb:kb + 1])

            if kb == 0:
                q_side(0)
                q_side(1)
            if kb + QA < NKB:
                q_side(kb + QA)

            # stage 3, one k-block behind so its matmuls never wait on the
            # scalar/vector producers of vk/qx at the head of the PE queue
            if kb >= 1:
                stage3(kb - 1)

            # weight prefetches at the bottom of the iteration: the slot's
            # same-ring readers are already emitted, so the Tile WAR gate
            # holds each DMA until they complete (no prologue flood, no
            # clobbering of the weights this iteration is reading)
            if kb + 2 < NKB:
                nc.sync.dma_start(wv_s[(kb + 2) % WVD], wv_d[kb + 2])
            if kb + 3 < NKB:
                nc.gpsimd.dma_start(wq_s[(kb + 3) % WQD], wq_d[kb + 3])

        # final k-block: accumulators evict (+h_bias) as their bank closes
        stage3(NKB - 1, evict=True)

    nc.compile()
    return nc


def _prep_host(inputs):
    bf16 = mybir.dt.np(BF16)
    v = np.asarray(inputs["v"], dtype=np.float32)
    q = np.asarray(inputs["q"], dtype=np.float32)
    v_w = np.asarray(inputs["v_w"], dtype=np.float32)
    q_w = np.asarray(inputs["q_w"], dtype=np.float32)
    v_g = float(np.asarray(inputs["v_g"], dtype=np.float32))
    q_g = float(np.asarray(inputs["q_g"], dtype=np.float32))
    v_b = np.asarray(inputs["v_b"], dtype=np.float32)
    q_b = np.asarray(inputs["q_b"], dtype=np.float32)
    h_mat = np.asarray(inputs["h_mat"], dtype=np.float32)
    h_bias = np.asarray(inputs["h_bias"], dtype=np.float32)

    s_v = v_g / float(np.linalg.norm(v_w))
    s_q = q_g / float(np.linalg.norm(q_w))

    wv_r = np.ascontiguousarray(
        (s_v * v_w).astype(bf16)
        .reshape(NKB, KB, TV, 128).transpose(0, 3, 2, 1)
        .reshape(NKB, 128, TV * KB))
    wq_r = np.ascontiguousarray(
        (s_q * q_w).astype(bf16)
        .reshape(NKB, KB, TQ, 128).transpose(0, 3, 2, 1)
        .reshape(NKB, 128, TQ * KB))
    hm = h_mat[0, :, 0, :]                       # [H, K]
    msb = np.ascontiguousarray(
        hm.T.reshape(NKB, 128, H).transpose(1, 0, 2)
        .reshape(128, NKB * H))
    vb_r = np.ascontiguousarray(v_b.reshape(NKB, 128).T)
    qb_r = np.ascontiguousarray(q_b.reshape(NKB, 128).T)
    hb = h_bias[0, :, 0, 0]                      # [H]
    bias = np.ascontiguousarray(np.broadcast_to(
        np.tile(np.repeat(hb, NQ), 2)[None, :], (128, 2 * H * NQ)))

    in_maps = []
    for c in range(N_CORES):
        vc = v[c * B_LOC:(c + 1) * B_LOC].astype(bf16)
        qc = q[c * B_LOC:(c + 1) * B_LOC].astype(bf16)
        vt_c = np.ascontiguousarray(
            vc.reshape(B_LOC, NV, TV, 128).transpose(2, 3, 0, 1)
            .reshape(TV // 4, 4, 128, MV).transpose(0, 2, 1, 3)
            .reshape(TV // 4, 128, 4 * MV))
        qt_c = np.ascontiguousarray(
            qc.reshape(B_LOC, NQ, TQ, 128).transpose(2, 3, 0, 1)
            .reshape(TQ // 4, 4, 128, MQ).transpose(0, 2, 1, 3)
            .reshape(TQ // 4, 128, 4 * MQ))
        in_maps.append({
            "vt": vt_c, "qt": qt_c, "wv": wv_r, "wq": wq_r,
            "msb": msb, "vb": vb_r, "qb": qb_r, "bias": bias,
        })
    return in_maps


def _run(inputs, trace=False):
    if "nc" not in _CACHE:
        _CACHE["nc"] = _build_program()
    nc = _CACHE["nc"]
    in_maps = _prep_host(inputs)
    res = run_bass_kernel_spmd(nc, in_maps, list(range(N_CORES)), trace=trace)
    out = np.empty((B, H, NV, NQ), dtype=np.float32)
    for c in range(N_CORES):
        oc = np.asarray(res.results[c]["out"], dtype=np.float32)
        out[c * B_LOC:(c + 1) * B_LOC] = (
            oc.reshape(4, NV, 2, H, NQ).transpose(0, 2, 3, 1, 4)
            .reshape(B_LOC, H, NV, NQ))
    return out, res


def kernel(**inputs):
    return _run(inputs)[0]


# revision 25
# speedup vs baseline: 1.0151x; 1.0133x over previous
"""Trainium2 Bass kernel for nn_BC_5274219839877.

Computes, for b=64, n_v=128, n_q=32, d_v=2048, d_q=1024, K=3072, H=8:
    v_ = relu((v_g/||v_w||) * v @ v_w^T + v_b)        [b, n_v, K]
    q_ = relu((q_g/||q_w||) * q @ q_w^T + q_b)        [b, n_q, K]
    out[b,h,i,j] = sum_k hm[h,k] v_[b,i,k] q_[b,j,k] + h_bias[h]

Sharding: data-parallel over batch across 8 NeuronCores (8 batches/core),
weights replicated.  All matmul operands are bf16 (norm-scales folded into
the weights host-side); the PE streams 1 column/cycle regardless of dtype,
so bf16 buys DMA bandwidth and removes the f32->f32r conversion copies.
The per-core PE floor is 491,520 column-cycles (~205us at 2.4GHz); the
steady-state stream here runs gapless at 216ns per 512-col matmul (silicon
peak), so the remaining wall time is the DMA-bound prologue (covered by
HAM-warm dummy matmuls), boot, and the eviction tail.

Key structure:
 - Stage-3 partial products accumulate directly in PSUM across all 24
   k-blocks (2 batch accumulators per 2KB bank; start=True only on each
   bank's first write - the whole-bank has_written clear happens once and
   the second accumulator's first matmul overwrites-where-clear).
 - The q side runs 2 k-blocks ahead; stage 3 lags 1 k-block so its
   matmuls never head-of-line-block the PE queue on scalar/vector
   producers.
 - Weight prefetch DMAs are emitted at the BOTTOM of the iteration with
   minimal ring depths, so Tile's WAR gate (against already-emitted
   readers) paces them and they never flood the prologue wire.
"""

import numpy as np

import concourse.bass as bass
import concourse.tile as tile
from concourse import bacc, mybir
from concourse.bass_utils import run_bass_kernel_spmd

F32 = mybir.dt.float32
BF16 = mybir.dt.bfloat16

N_CORES = 8
B = 64
B_LOC = B // N_CORES       # 8 batches per core
NV = 128
NQ = 32
DV = 2048
DQ = 1024
K = 3072
H = 8

KB = 128                   # k-block size (PSUM partition dim)
NKB = K // KB              # 24 k-blocks
TV = DV // 128             # 16 d-tiles (v side)
TQ = DQ // 128             # 8 d-tiles (q side)
MV = B_LOC * NV            # 1024
MQ = B_LOC * NQ            # 256

WARM = 118                 # HAM-warm dummy matmuls covering the DMA prologue
QA = 2                     # q-side runs QA k-blocks ahead of v-side
QXD = 4                    # qx ring depth
WVD = 2                    # wv ring depth (bottom-emitted => WAR-gated)
WQD = 3                    # wq ring depth (same)

_CACHE = {}


def _build_program():
    nc = bacc.Bacc("TRN2", target_bir_lowering=False, debug=False,
                   num_devices=N_CORES)

    vt_d = nc.dram_tensor("vt", [TV // 4, 128, 4 * MV], BF16,
                          kind="ExternalInput")
    qt_d = nc.dram_tensor("qt", [TQ // 4, 128, 4 * MQ], BF16,
                          kind="ExternalInput")
    wv_d = nc.dram_tensor("wv", [NKB, 128, TV * KB], BF16,
                          kind="ExternalInput")
    wq_d = nc.dram_tensor("wq", [NKB, 128, TQ * KB], BF16,
                          kind="ExternalInput")
    msb_d = nc.dram_tensor("msb", [128, NKB * H], F32, kind="ExternalInput")
    vb_d = nc.dram_tensor("vb", [128, NKB], F32, kind="ExternalInput")
    qb_d = nc.dram_tensor("qb", [128, NKB], F32, kind="ExternalInput")
    bias_d = nc.dram_tensor("bias", [128, 2 * H * NQ], BF16,
                            kind="ExternalInput")
    out_d = nc.dram_tensor("out", [B_LOC // 2, 128, 2 * H * NQ], BF16,
                           kind="ExternalOutput")

    relu = mybir.ActivationFunctionType.Relu

    msb = nc.alloc_sbuf_tensor("msb_s", [128, NKB * H], F32).ap()
    vb = nc.alloc_sbuf_tensor("vb_s", [128, NKB], F32).ap()
    qb = nc.alloc_sbuf_tensor("qb_s", [128, NKB], F32).ap()
    bias = nc.alloc_sbuf_tensor("bias_s", [128, 2 * H * NQ], BF16).ap()
    vt_big = nc.alloc_sbuf_tensor("vts", [128, TV * MV], BF16).ap()
    vt = [vt_big[:, t * MV:(t + 1) * MV] for t in range(TV)]
    qt_big = nc.alloc_sbuf_tensor("qts", [128, TQ * MQ], BF16).ap()
    qt = [qt_big[:, t * MQ:(t + 1) * MQ] for t in range(TQ)]
    wv_s = [nc.alloc_sbuf_tensor(f"wvs{i}", [128, TV * KB], BF16).ap()
            for i in range(WVD)]
    wq_s = [nc.alloc_sbuf_tensor(f"wqs{i}", [128, TQ * KB], BF16).ap()
            for i in range(WQD)]
    vk = [nc.alloc_sbuf_tensor(f"vk{i}", [128, MV], BF16).ap()
          for i in range(2)]
    qk = [nc.alloc_sbuf_tensor(f"qk{i}", [128, MQ], BF16).ap()
          for i in range(2)]
    qx = [nc.alloc_sbuf_tensor(f"qx{i}", [128, H * MQ], BF16).ap()
          for i in range(QXD)]
    outs = [nc.alloc_sbuf_tensor(f"outs{i}", [128, 2 * H * NQ], BF16).ap()
            for i in range(B_LOC // 2)]
    warm = nc.alloc_sbuf_tensor("warm", [128, 256], BF16).ap()

    psv = [nc.alloc_psum_tensor(f"psv{i}", [128, 512], F32).ap()
           for i in range(3)]
    psq = nc.alloc_psum_tensor("psq", [128, MQ], F32).ap()
    ps3 = [nc.alloc_psum_tensor(f"ps3{i}", [128, 2 * H * NQ], F32).ap()
           for i in range(B_LOC // 2)]

    with tile.TileContext(nc) as tc:
        def vt_dma(c):
            return (vt_big[:, c * 4 * MV:(c + 1) * 4 * MV], vt_d[c])

        # Max ring parallelism: concurrent DMA rings raise aggregate
        # throughput (~500 GB/s); everything lands together ~20us in and
        # the warm matmuls cover the window.
        # Max ring parallelism: concurrent DMA rings raise aggregate
        # throughput (~500 GB/s); everything lands together ~20us in and
        # the warm matmuls cover the window.
        nc.vector.memset(warm, 0.0)
        nc.sync.dma_start(wv_s[0], wv_d[0])
        nc.scalar.dma_start(*vt_dma(1))
        nc.gpsimd.dma_start(*vt_dma(3))
        nc.sync.dma_start(*vt_dma(0))
        nc.scalar.dma_start(*vt_dma(2))
        nc.gpsimd.dma_start(vb, vb_d.ap())
        nc.gpsimd.dma_start(qb, qb_d.ap())
        for c in range(TQ // 4):
            nc.sync.dma_start(qt_big[:, c * 4 * MQ:(c + 1) * 4 * MQ],
                              qt_d[c])
        for i in range(WQD):
            nc.gpsimd.dma_start(wq_s[i], wq_d[i])
        nc.sync.dma_start(wv_s[1], wv_d[1])
        nc.gpsimd.dma_start(msb, msb_d.ap())
        nc.gpsimd.dma_start(bias, bias_d.ap())

        # HAM pre-warm: dummy matmuls on zeros while the prologue DMAs
        # stream, so the real matmul stream starts at the warm PE clock
        for i in range(WARM):
            nc.tensor.matmul(psv[0][:, :256], warm[:, :128], warm,
                             start=True, stop=True)

        def q_side(kb):
            # q_: qk[k, (b,j)] = relu(qt @ wq + qb); qx[k,(b,h,j)] = hm*qk
            # (one PSUM bank: the chain and its activation never overlap
            # the next k-block's chain, 8.7us apart)
            qps = psq
            wqb = wq_s[kb % WQD]
            for t in range(TQ):
                nc.tensor.matmul(
                    qps[:],
                    wqb[:, t * KB:(t + 1) * KB],
                    qt[t],
                    start=(t == 0), stop=(t == TQ - 1))
            qkb = qk[kb % 2]
            nc.scalar.activation(qkb[:], qps[:], relu,
                                 bias=qb[:, kb:kb + 1])
            qxb = qx[kb % QXD]
            qx4 = qxb.rearrange("p (b h j) -> p b h j", b=B_LOC, h=H)
            qk3 = qkb.rearrange("p (b j) -> p b j", b=B_LOC)
            for h in range(H):
                nc.vector.tensor_scalar_mul(
                    qx4[:, :, h, :], qk3[:, :, :],
                    msb[:, kb * H + h:kb * H + h + 1])

        def stage3(kb, evict=False):
            # ps3[b//2][:, (b%2)*256:...] += vk[:, b].T @ qx[:, b, :, :]
            vkb = vk[kb % 2]
            qxb = qx[kb % QXD]
            for b_ in range(B_LOC):
                nc.tensor.matmul(
                    ps3[b_ // 2][:, (b_ % 2) * H * NQ:(b_ % 2 + 1) * H * NQ],
                    vkb[:, b_ * NV:(b_ + 1) * NV],
                    qxb[:, b_ * H * NQ:(b_ + 1) * H * NQ],
                    start=(kb == 0 and b_ % 2 == 0),
                    stop=(kb == NKB - 1),
                    skip_group_check=True)
                if evict and b_ % 2 == 1:
                    p = b_ // 2
                    nc.vector.tensor_add(outs[p][:], ps3[p][:], bias[:])
                    nc.sync.dma_start(out_d[p], outs[p][:])

        # --- k-blocked fused pipeline ---
        for kb in range(NKB):
            wvb = wv_s[kb % WVD]

            # stage 1 (v): vk[k, m] = relu((v @ v_w^T)^T + v_b)
            vkb = vk[kb % 2]
            for mc in range(MV // 512):
                ps = psv[(2 * kb + mc) % 3]
                for t in range(TV):
                    nc.tensor.matmul(
                        ps[:],
                        wvb[:, t * KB:(t + 1) * KB],
                        vt[t][:, mc * 512:(mc + 1) * 512],
                        start=(t == 0), stop=(t == TV - 1))
                nc.scalar.activation(
                    vkb[:, mc * 512:(mc + 1) * 512], ps[:], relu,
                    bias=vb[:, kb:kb + 1])

            if kb == 0:
                q_side(0)
                q_side(1)
            if kb + QA < NKB:
                q_side(kb + QA)

            # stage 3, one k-block behind so its matmuls never wait on the
            # scalar/vector producers of vk/qx at the head of the PE queue
            if kb >= 1:
                stage3(kb - 1)

            # weight prefetches at the bottom of the iteration: the slot's
            # same-ring readers are already emitted, so the Tile WAR gate
            # holds each DMA until they complete (no prologue flood, no
            # clobbering of the weights this iteration is reading)
            if kb + 2 < NKB:
                nc.sync.dma_start(wv_s[(kb + 2) % WVD], wv_d[kb + 2])
            if kb + 3 < NKB:
                nc.gpsimd.dma_start(wq_s[(kb + 3) % WQD], wq_d[kb + 3])

        # final k-block: accumulators evict (+h_bias) as their bank closes
        stage3(NKB - 1, evict=True)

    nc.compile()
    return nc


def _prep_host(inputs):
    bf16 = mybir.dt.np(BF16)
    v = np.asarray(inputs["v"], dtype=np.float32)
    q = np.asarray(inputs["q"], dtype=np.float32)
    v_w = np.asarray(inputs["v_w"], dtype=np.float32)
    q_w = np.asarray(inputs["q_w"], dtype=np.float32)
    v_g = float(np.asarray(inputs["v_g"], dtype=np.float32))
    q_g = float(np.asarray(inputs["q_g"], dtype=np.float32))
    v_b = np.asarray(inputs["v_b"], dtype=np.float32)
    q_b = np.asarray(inputs["q_b"], dtype=np.float32)
    h_mat = np.asarray(inputs["h_mat"], dtype=np.float32)
    h_bias = np.asarray(inputs["h_bias"], dtype=np.float32)

    s_v = v_g / float(np.linalg.norm(v_w))
    s_q = q_g / float(np.linalg.norm(q_w))

    wv_r = np.ascontiguousarray(
        (s_v * v_w).astype(bf16)
        .reshape(NKB, KB, TV, 128).transpose(0, 3, 2, 1)
        .reshape(NKB, 128, TV * KB))
    wq_r = np.ascontiguousarray(
        (s_q * q_w).astype(bf16)
        .reshape(NKB, KB, TQ, 128).transpose(0, 3, 2, 1)
        .reshape(NKB, 128, TQ * KB))
    hm = h_mat[0, :, 0, :]                       # [H, K]
    msb = np.ascontiguousarray(
        hm.T.reshape(NKB, 128, H).transpose(1, 0, 2)
        .reshape(128, NKB * H))
    vb_r = np.ascontiguousarray(v_b.reshape(NKB, 128).T)
    qb_r = np.ascontiguousarray(q_b.reshape(NKB, 128).T)
    hb = h_bias[0, :, 0, 0]                      # [H]
    bias = np.ascontiguousarray(np.broadcast_to(
        np.tile(np.repeat(hb, NQ), 2)[None, :],
        (128, 2 * H * NQ)).astype(bf16))

    in_maps = []
    for c in range(N_CORES):
        vc = v[c * B_LOC:(c + 1) * B_LOC].astype(bf16)
        qc = q[c * B_LOC:(c + 1) * B_LOC].astype(bf16)
        vt_c = np.ascontiguousarray(
            vc.reshape(B_LOC, NV, TV, 128).transpose(2, 3, 0, 1)
            .reshape(TV // 4, 4, 128, MV).transpose(0, 2, 1, 3)
            .reshape(TV // 4, 128, 4 * MV))
        qt_c = np.ascontiguousarray(
            qc.reshape(B_LOC, NQ, TQ, 128).transpose(2, 3, 0, 1)
            .reshape(TQ // 4, 4, 128, MQ).transpose(0, 2, 1, 3)
            .reshape(TQ // 4, 128, 4 * MQ))
        in_maps.append({
            "vt": vt_c, "qt": qt_c, "wv": wv_r, "wq": wq_r,
            "msb": msb, "vb": vb_r, "qb": qb_r, "bias": bias,
        })
    return in_maps


def _run(inputs, trace=False):
    if "nc" not in _CACHE:
        _CACHE["nc"] = _build_program()
    nc = _CACHE["nc"]
    in_maps = _prep_host(inputs)
    res = run_bass_kernel_spmd(nc, in_maps, list(range(N_CORES)), trace=trace)
    out = np.empty((B, H, NV, NQ), dtype=np.float32)
    for c in range(N_CORES):
        oc = np.asarray(res.results[c]["out"], dtype=np.float32)
        out[c * B_LOC:(c + 1) * B_LOC] = (
            oc.reshape(4, NV, 2, H, NQ).transpose(0, 2, 3, 1, 4)
            .reshape(B_LOC, H, NV, NQ))
    return out, res


def kernel(**inputs):
    return _run(inputs)[0]
